# revision 1
# baseline (speedup 1.0000x reference)
"""Alignment kernel (decomposable-attention style) for Trainium2.

Per batch element (one NeuronCore, data-parallel over B=8):
    at_a = relu(a @ W + bias) * temp      (temp folded into at_a)
    at_b = relu(b @ W + bias)
    E    = exp(at_a @ at_b.T)             [La, Lb]; softmax is shift-invariant
                                          and scores are O(3), so no max pass
    feature_a = (E / rowsum(E))  @ b      -> [La, D]
    feature_b = (E / colsum(E)).T @ a     -> [Lb, D]

Two flash-style passes; each pass produces one feature's unnormalized PSUM
accumulation and the OTHER feature's softmax denominator via DVE row
reductions of the exp tiles.
"""

import sys

if "/opt/trn_rl_repo" not in sys.path:
    sys.path.insert(0, "/opt/trn_rl_repo")

import ml_dtypes
import numpy as np

import concourse.bass as bass
import concourse.mybir as mybir
from concourse.tile import TileContext
from concourse.vector_clock import ScopedClock, VectorClock
from concourse.bass_utils import run_bass_kernel_spmd

# Problem constants (hardcoded per harness contract)
B, L, D = 8, 2048, 256
P = 128          # SBUF partitions
KD = D // P      # 2 contraction chunks over D
NL = L // P      # 16 row chunks
F = 512          # score-tile free dim (one fp32 PSUM bank)
NS = L // F      # 4 super chunks

FP32 = mybir.dt.float32
RELU = mybir.ActivationFunctionType.Relu
EXP = mybir.ActivationFunctionType.Exp

# matmul-operand dtype: "bf16" (fast LDWEIGHTS, half DMA) or "fp32r"
MM_DTYPE = "bf16"
STRIP_EPILOGUE = True


class SplitDrainTileContext(TileContext):
    """The walrus build in this container only accepts a single sync-wait
    per CTRL instruction; stock Tile emits one epilogue Drain waiting on
    every active processor.  Emit one single-wait Drain per processor
    instead (same semantics: SP observes every proc's final tick before
    the exit barrier)."""

    def _drain_and_barrier(self, tick_clock, wait_clock):
        gc = tick_clock.global_clock
        n = len(gc)
        for proc in range(n):
            tick = gc[proc]
            if tick <= 0:
                continue
            vc = VectorClock([0] * n)
            vc.require_at_least(proc, tick)
            drain_inst = self.nc.sync.drain()
            wait_clock.add_sem_waits(drain_inst.ins, ScopedClock({None: vc}))
        if STRIP_EPILOGUE:
            # outputs are complete once the split drains retire; sems are
            # reset by NRT on (re)load and each PJRT dispatch loads fresh
            popped = self.nc._tile_sem_poison_stack.pop()
            assert popped is self._sem_poison
            return
        self.nc.all_engine_barrier(sem_only=True)
        assert self.sems is not None
        popped = self.nc._tile_sem_poison_stack.pop()
        assert popped is self._sem_poison
        self.nc.clear_and_free_semaphores(list(self.sems.allocated().values()))
        self.nc.all_engine_barrier(sem_only=True)


def split_multiwaits(nc):
    """This container's walrus accepts only ONE sync-wait per instruction.
    Hoist extra waits onto same-engine NoOps immediately preceding the
    instruction (engine streams are in-order, so semantics are identical)."""
    ctr = 0
    for fn in nc.m.functions:
        for blk in fn.blocks:
            out = []
            for inst in blk.instructions:
                si = inst.sync_info
                if si is not None and si.on_wait and len(si.on_wait) > 1:
                    waits = list(si.on_wait)
                    for w in waits[:-1]:
                        nop = mybir.InstNoOp(name=f"wsplit_{ctr}", ins=[], outs=[])
                        ctr += 1
                        nop.engine = inst.engine
                        nop.sync_info = mybir.SyncInfo(on_wait=[w], on_update=[])
                        out.append(nop)
                    inst.sync_info = mybir.SyncInfo(
                        on_wait=[waits[-1]], on_update=list(si.on_update)
                    )
                out.append(inst)
            blk.instructions = out


def build_kernel(mm_dtype=None):
    mm_dtype = mm_dtype or MM_DTYPE
    if mm_dtype == "bf16":
        MMDT = mybir.dt.bfloat16
        cast_on_device = False   # host feeds bf16 directly
    else:
        MMDT = mybir.dt.float32r
        cast_on_device = True    # gpsimd DMA rounds fp32 -> fp32r

    IN_DT = FP32 if cast_on_device else MMDT
    nc = bass.Bass()

    aT_d = nc.dram_tensor("aT", [D, L], IN_DT, kind="ExternalInput")
    bT_d = nc.dram_tensor("bT", [D, L], IN_DT, kind="ExternalInput")
    a_d = nc.dram_tensor("a_nat", [L, D], IN_DT, kind="ExternalInput")
    b_d = nc.dram_tensor("b_nat", [L, D], IN_DT, kind="ExternalInput")
    w_d = nc.dram_tensor("w", [D, D], IN_DT, kind="ExternalInput")
    bias_d = nc.dram_tensor("bias", [D, 1], FP32, kind="ExternalInput")
    temp_d = nc.dram_tensor("temp", [1, 1], FP32, kind="ExternalInput")
    fa_d = nc.dram_tensor("feature_a", [L, D], FP32, kind="ExternalOutput")
    fb_d = nc.dram_tensor("feature_b", [L, D], FP32, kind="ExternalOutput")

    # DRAM views for chunked access
    aT_v = aT_d[:].rearrange("(kc p) l -> p kc l", p=P)      # [128, KD, L]
    bT_v = bT_d[:].rearrange("(kc p) l -> p kc l", p=P)
    a_v = a_d[:].rearrange("(n p) d -> p n d", p=P)          # [128, NL, D]
    b_v = b_d[:].rearrange("(n p) d -> p n d", p=P)
    w_v = w_d[:].rearrange("(kc p) n -> p kc n", p=P)        # [128, KD, D]
    bias_v = bias_d[:].rearrange("(c p) one -> p c one", p=P)
    fa_v = fa_d[:].rearrange("(n p) d -> p n d", p=P)
    fb_v = fb_d[:].rearrange("(n p) d -> p n d", p=P)

    dma_in = nc.gpsimd if cast_on_device else nc.sync

    with SplitDrainTileContext(nc) as tc:
        with (
            tc.tile_pool(name="consts", bufs=1) as consts,
            tc.tile_pool(name="bigbuf", bufs=1) as bigbuf,
            tc.tile_pool(name="etile", bufs=6) as etile,
            tc.tile_pool(name="ps_s", bufs=4, space="PSUM") as ps_s_pool,
            tc.tile_pool(name="ps_f", bufs=1, space="PSUM") as ps_f_pool,
            tc.tile_pool(name="warm", bufs=1) as warm_pool,
        ):
            # ---- PE warmup: ~5us of dummy matmuls so the HAM clock-gate
            #      opens (K=8/8) before the real stream begins ----
            wsrc = warm_pool.tile([P, P], MMDT)
            nc.vector.memset(wsrc[:], 0.0)
            # preload the exp/relu ACT table set while ACT is idle
            wact = warm_pool.tile([P, 1], FP32)
            nc.scalar.activation(out=wact[:], in_=wsrc[:, 0:1], func=EXP)
            ps_w = ps_s_pool.tile([P, F], FP32, name="ps_w", tag="ps")
            for _ in range(48):
                nc.tensor.matmul(ps_w[:, :P], lhsT=wsrc[:], rhs=wsrc[:],
                                 start=True, stop=True)

            # ---- constants ----
            w_sb = consts.tile([P, KD, D], MMDT)
            nc.gpsimd.dma_start(out=w_sb[:], in_=w_v)
            bias_sb = consts.tile([P, KD], FP32)
            nc.gpsimd.dma_start(out=bias_sb[:], in_=bias_v[:, :, 0])
            temp_sb = consts.tile([P, 1], FP32)
            nc.gpsimd.dma_start(out=temp_sb[:], in_=temp_d[:].to_broadcast([P, 1]))
            # bias scaled by temperature (for the at_a branch)
            bias_t_sb = consts.tile([P, KD], FP32)
            nc.vector.tensor_scalar_mul(
                out=bias_t_sb[:], in0=bias_sb[:], scalar1=temp_sb[:, 0:1]
            )

            # ---- big SBUF residents ----
            aT_sb = bigbuf.tile([P, KD, L], MMDT)
            bT_sb = bigbuf.tile([P, KD, L], MMDT)
            a_sb = bigbuf.tile([P, NL, D], MMDT)
            b_sb = bigbuf.tile([P, NL, D], MMDT)
            at_a = bigbuf.tile([P, KD, L], MMDT)   # temp * relu(aW + bias)
            at_b = bigbuf.tile([P, KD, L], MMDT)   # relu(bW + bias)
            fa_un = bigbuf.tile([P, NL, D], FP32)  # unnormalized feature_a
            fa_st = bigbuf.tile([P, NL, D], FP32)  # normalized staging
            fb_st = bigbuf.tile([P, NL, D], FP32)
            colsum_p = bigbuf.tile([P, NL, NS], FP32)
            rowsum_p = bigbuf.tile([P, NL, NS], FP32)
            inv_col = bigbuf.tile([P, NL], FP32)
            inv_row = bigbuf.tile([P, NL], FP32)

            # input loads, sliced, alternating across the TWO hardware DGE
            # queues (SP and Activation); all aT first so dense-a starts
            # after the first kc pair, then bT, then the PV operands
            # aT/bT in 2KB-line slices alternating across both HW queues,
            # kc-pairs together so dense can start on the first pair
            # three issue engines: SP-HWDGE, ACT-HWDGE, gpsimd-SWDGE.
            # The scalar queue only gets EARLY slices: its issue instructions
            # must all retire before the first dense relu enters ACT's FIFO.
            eng3 = [dma_in, nc.scalar, nc.gpsimd] if not cast_on_device else [dma_in] * 3
            W2 = 2 * F
            qi = 0
            for hf in range(L // W2):
                sl = slice(hf * W2, (hf + 1) * W2)
                for kc in range(KD):
                    eng3[qi % 3].dma_start(out=bT_sb[:, kc, sl], in_=bT_v[:, kc, sl])
                    qi += 1
            for hf in range(L // W2):
                sl = slice(hf * W2, (hf + 1) * W2)
                for kc in range(KD):
                    eng3[qi % 3].dma_start(out=aT_sb[:, kc, sl], in_=aT_v[:, kc, sl])
                    qi += 1
            for ns in range(NS):
                sl = slice(ns * (NL // NS), (ns + 1) * (NL // NS))
                (dma_in if ns % 2 == 0 else nc.gpsimd).dma_start(
                    out=b_sb[:, sl, :], in_=b_v[:, sl, :])
                # a_nat isn't needed until pass 2 (~halfway in)
                nc.gpsimd.dma_start(out=a_sb[:, sl, :], in_=a_v[:, sl, :])

            # ---- phase 1: dense + relu ----
            def dense_block(src_sb, dst, ls, scaled):
                sl = slice(ls * F, (ls + 1) * F)
                for dout in range(KD):
                    wcol = slice(dout * P, (dout + 1) * P)
                    ps = ps_s_pool.tile([P, F], FP32, name="ps", tag="ps")
                    for kc in range(KD):
                        nc.tensor.matmul(
                            ps[:],
                            lhsT=w_sb[:, kc, wcol],
                            rhs=src_sb[:, kc, sl],
                            start=(kc == 0),
                            stop=(kc == KD - 1),
                        )
                    if scaled:
                        nc.scalar.activation(
                            out=dst[:, dout, sl], in_=ps[:], func=RELU,
                            bias=bias_t_sb[:, dout : dout + 1],
                            scale=temp_sb[:, 0:1],
                        )
                    else:
                        # relu(x + bias) fused on the (idle) vector engine so
                        # dense-b isn't paced by ACT evictions at startup
                        nc.vector.tensor_scalar(
                            out=dst[:, dout, sl], in0=ps[:],
                            scalar1=bias_sb[:, dout : dout + 1], scalar2=0.0,
                            op0=mybir.AluOpType.add, op1=mybir.AluOpType.max,
                        )

            for ls in range(NS):
                dense_block(bT_sb, at_b, ls, False)

            # ---- phase 2 (pass 1): ET tiles [m, la] -> feature_a accum +
            #      colsum(E) partials (DVE row-reduce over la) ----
            for ls in range(NS):
                dense_block(aT_sb, at_a, ls, True)
                la_sl = slice(ls * F, (ls + 1) * F)
                ps_feat = [
                    ps_f_pool.tile([P, D], FP32, name=f"psfa{ls}_{j}", tag=f"psf{j}")
                    for j in range(4)
                ]
                prev = None
                for mc in range(NL):
                    m_sl = slice(mc * P, (mc + 1) * P)
                    ps = ps_s_pool.tile([P, F], FP32, name="ps", tag="ps")
                    for kc in range(KD):
                        nc.tensor.matmul(
                            ps[:],
                            lhsT=at_b[:, kc, m_sl],
                            rhs=at_a[:, kc, la_sl],
                            start=(kc == 0),
                            stop=(kc == KD - 1),
                        )
                    et = etile.tile([P, F], MMDT, name="et", tag="et")
                    nc.scalar.activation(out=et[:], in_=ps[:], func=EXP)
                    nc.vector.tensor_reduce(
                        out=colsum_p[:, mc, ls : ls + 1], in_=et[:],
                        axis=mybir.AxisListType.X, op=mybir.AluOpType.add,
                    )
                    if prev is not None:
                        pet, pmc = prev
                        for j in range(4):
                            nc.tensor.matmul(
                                ps_feat[j][:],
                                lhsT=pet[:, j * P : (j + 1) * P],
                                rhs=b_sb[:, pmc, :],
                                start=(pmc == 0),
                                stop=False,
                            )
                    prev = (et, mc)
                pet, pmc = prev
                for j in range(4):
                    nc.tensor.matmul(
                        ps_feat[j][:],
                        lhsT=pet[:, j * P : (j + 1) * P],
                        rhs=b_sb[:, pmc, :],
                        start=False,
                        stop=True,
                    )
                with tc.high_priority():
                    for j in range(4):
                        nc.vector.tensor_copy(
                            out=fa_un[:, ls * 4 + j, :], in_=ps_feat[j][:]
                        )

            # feature_b normalizer
            nc.vector.tensor_reduce(
                out=inv_col[:], in_=colsum_p[:], axis=mybir.AxisListType.X,
                op=mybir.AluOpType.add,
            )
            nc.vector.reciprocal(out=inv_col[:], in_=inv_col[:])

            # ---- phase 3 (pass 2): E tiles [la, m] -> feature_b +
            #      rowsum(E) partials ----
            for ms in range(NS):
                m_sl = slice(ms * F, (ms + 1) * F)
                ps_feat = [
                    ps_f_pool.tile([P, D], FP32, name=f"psfb{ms}_{j}", tag=f"psf{j}")
                    for j in range(4)
                ]
                prev = None
                for lc in range(NL):
                    la_sl = slice(lc * P, (lc + 1) * P)
                    ps = ps_s_pool.tile([P, F], FP32, name="ps", tag="ps")
                    for kc in range(KD):
                        nc.tensor.matmul(
                            ps[:],
                            lhsT=at_a[:, kc, la_sl],
                            rhs=at_b[:, kc, m_sl],
                            start=(kc == 0),
                            stop=(kc == KD - 1),
                        )
                    e = etile.tile([P, F], MMDT, name="et", tag="et")
                    nc.scalar.activation(out=e[:], in_=ps[:], func=EXP)
                    nc.vector.tensor_reduce(
                        out=rowsum_p[:, lc, ms : ms + 1], in_=e[:],
                        axis=mybir.AxisListType.X, op=mybir.AluOpType.add,
                    )
                    if prev is not None:
                        pe_t, plc = prev
                        for j in range(4):
                            nc.tensor.matmul(
                                ps_feat[j][:],
                                lhsT=pe_t[:, j * P : (j + 1) * P],
                                rhs=a_sb[:, plc, :],
                                start=(plc == 0),
                                stop=False,
                            )
                    prev = (e, lc)
                    if ms == NS - 1:
                        # rowsum(E) for row-chunk lc is now complete: finish
                        # feature_a for lc while the PE keeps streaming
                        nc.vector.tensor_reduce(
                            out=inv_row[:, lc : lc + 1], in_=rowsum_p[:, lc, :],
                            axis=mybir.AxisListType.X, op=mybir.AluOpType.add,
                        )
                        nc.vector.reciprocal(
                            out=inv_row[:, lc : lc + 1], in_=inv_row[:, lc : lc + 1]
                        )
                        nc.vector.tensor_scalar_mul(
                            out=fa_st[:, lc, :], in0=fa_un[:, lc, :],
                            scalar1=inv_row[:, lc : lc + 1],
                        )
                        if lc < NL - 4:
                            if lc % 2 == 1:
                                po = lc - 1
                                nc.scalar.dma_start(
                                    out=fa_v[:, po : po + 2, :],
                                    in_=fa_st[:, po : po + 2, :],
                                )
                        else:
                            eng_o = nc.scalar if lc % 2 else nc.sync
                            eng_o.dma_start(
                                out=fa_v[:, lc : lc + 1, :],
                                in_=fa_st[:, lc : lc + 1, :],
                            )
                pe_t, plc = prev
                for j in range(4):
                    nc.tensor.matmul(
                        ps_feat[j][:],
                        lhsT=pe_t[:, j * P : (j + 1) * P],
                        rhs=a_sb[:, plc, :],
                        start=False,
                        stop=True,
                    )
                # normalize + stage + paired DMAs on alternating queues
                for j in range(4):
                    mc_out = ms * 4 + j
                    with tc.high_priority():
                        nc.vector.tensor_scalar_mul(
                            out=fb_st[:, mc_out, :], in0=ps_feat[j][:],
                            scalar1=inv_col[:, mc_out : mc_out + 1],
                        )
                    if ms == NS - 1:
                        mc_out = ms * 4 + j
                        eng_o = nc.sync if j % 2 else nc.scalar
                        eng_o.dma_start(
                            out=fb_v[:, mc_out : mc_out + 1, :],
                            in_=fb_st[:, mc_out : mc_out + 1, :],
                        )
                    elif j % 2 == 1:
                        po = ms * 4 + j - 1
                        nc.sync.dma_start(
                            out=fb_v[:, po : po + 2, :],
                            in_=fb_st[:, po : po + 2, :],
                        )

    split_multiwaits(nc)
    return nc


_NC_CACHE = {}


def make_in_maps(a, b, dense_w, dense_b, temp, mm_dtype=None):
    mm_dtype = mm_dtype or MM_DTYPE
    in_np_dt = ml_dtypes.bfloat16 if mm_dtype == "bf16" else np.float32
    w_arr = np.ascontiguousarray(dense_w.astype(in_np_dt))
    bias_arr = np.ascontiguousarray(dense_b.reshape(D, 1).astype(np.float32))
    temp_arr = np.array([[temp]], dtype=np.float32)
    in_maps = []
    for i in range(B):
        in_maps.append({
            "aT": np.ascontiguousarray(a[i].T.astype(in_np_dt)),
            "bT": np.ascontiguousarray(b[i].T.astype(in_np_dt)),
            "a_nat": np.ascontiguousarray(a[i].astype(in_np_dt)),
            "b_nat": np.ascontiguousarray(b[i].astype(in_np_dt)),
            "w": w_arr,
            "bias": bias_arr,
            "temp": temp_arr,
        })
    return in_maps


def run(a, b, dense_w, dense_b, temperature, mm_dtype=None, **spmd_kwargs):
    mm_dtype = mm_dtype or MM_DTYPE
    a = np.asarray(a, dtype=np.float32)
    b = np.asarray(b, dtype=np.float32)
    dense_w = np.asarray(dense_w, dtype=np.float32)
    dense_b = np.asarray(dense_b, dtype=np.float32)
    temp = np.float32(np.asarray(temperature).reshape(-1)[0])

    if mm_dtype not in _NC_CACHE:
        _NC_CACHE[mm_dtype] = build_kernel(mm_dtype)
    nc = _NC_CACHE[mm_dtype]

    in_maps = make_in_maps(a, b, dense_w, dense_b, temp, mm_dtype)
    res = run_bass_kernel_spmd(nc, in_maps, core_ids=list(range(B)), **spmd_kwargs)
    fa = np.stack([res.results[i]["feature_a"] for i in range(B)])
    fb = np.stack([res.results[i]["feature_b"] for i in range(B)])
    return fa, fb, res


def kernel(a, b, mask_a, mask_b, dense_w, dense_b, temperature, **_ignored):
    fa, fb, _ = run(a, b, dense_w, dense_b, temperature)
    return fa, fb


if __name__ == "__main__":
    rng = np.random.default_rng(0)
    a = rng.standard_normal((B, L, D), dtype=np.float32)
    b = rng.standard_normal((B, L, D), dtype=np.float32)
    w = (rng.standard_normal((D, D)) / 16).astype(np.float32)
    bias = np.zeros((D,), np.float32)
    fa, fb = kernel(a, b, None, None, w, bias, np.float32(1 / 16))
    print(fa.shape, fb.shape, fa.dtype)



# revision 2
# speedup vs baseline: 1.1762x; 1.1762x over previous
"""Alignment kernel (decomposable-attention style) for Trainium2.

Per batch element (one NeuronCore, data-parallel over B=8):
    at_a = relu(a @ (W*temp) + bias*temp)   (temp folded into W host-side)
    at_b = relu(b @ W + bias)
    E    = exp(at_a @ at_b.T)               [La, Lb]; softmax is shift-invariant
                                            and scores are O(3), so no max pass
    feature_a = (E / rowsum(E))  @ b        -> [La, D]
    feature_b = (E / colsum(E)).T @ a       -> [Lb, D]

Single-score-pass scheme: E tiles [la:128, m:512] are computed ONCE.
Per tile: feature_b accumulates via lhsT = E slices against rhs = [a|1]
(the ones column makes colsum fall out of the same matmul), and the tile
is PE-transposed (bf16, 4x 128x128 blocks) into an SBUF-resident ET.
Phase B sweeps ET for feature_a against rhs = [b|1] (rowsum free).
No DVE reductions anywhere; exp runs once instead of twice.
"""

import sys

if "/opt/trn_rl_repo" not in sys.path:
    sys.path.insert(0, "/opt/trn_rl_repo")

import ml_dtypes
import numpy as np

import concourse.bass as bass
import concourse.mybir as mybir
from concourse.masks import make_identity
from concourse.tile import TileContext
from concourse.vector_clock import ScopedClock, VectorClock
from concourse.bass_utils import run_bass_kernel_spmd

# Problem constants (hardcoded per harness contract)
B, L, D = 8, 2048, 256
P = 128          # SBUF partitions
KD = D // P      # 2 contraction chunks over D
NL = L // P      # 16 row chunks
F = 512          # score-tile free dim (one fp32 PSUM bank)
NS = L // F      # 4 super chunks
DO = D + 1       # feature rhs width: [a|1] / [b|1]

FP32 = mybir.dt.float32
BF16 = mybir.dt.bfloat16
RELU = mybir.ActivationFunctionType.Relu
EXP = mybir.ActivationFunctionType.Exp

STRIP_EPILOGUE = True


class SplitDrainTileContext(TileContext):
    """The walrus build in this container only accepts a single sync-wait
    per CTRL instruction; stock Tile emits one epilogue Drain waiting on
    every active processor.  Emit one single-wait Drain per processor
    instead (same semantics: SP observes every proc's final tick before
    the exit barrier)."""

    def _drain_and_barrier(self, tick_clock, wait_clock):
        gc = tick_clock.global_clock
        n = len(gc)
        for proc in range(n):
            tick = gc[proc]
            if tick <= 0:
                continue
            vc = VectorClock([0] * n)
            vc.require_at_least(proc, tick)
            drain_inst = self.nc.sync.drain()
            wait_clock.add_sem_waits(drain_inst.ins, ScopedClock({None: vc}))
        if STRIP_EPILOGUE:
            # outputs are complete once the split drains retire; sems are
            # reset by NRT on (re)load and each PJRT dispatch loads fresh
            popped = self.nc._tile_sem_poison_stack.pop()
            assert popped is self._sem_poison
            return
        self.nc.all_engine_barrier(sem_only=True)
        assert self.sems is not None
        popped = self.nc._tile_sem_poison_stack.pop()
        assert popped is self._sem_poison
        self.nc.clear_and_free_semaphores(list(self.sems.allocated().values()))
        self.nc.all_engine_barrier(sem_only=True)


def split_multiwaits(nc):
    """This container's walrus accepts only ONE sync-wait per instruction.
    Hoist extra waits onto same-engine NoOps immediately preceding the
    instruction (engine streams are in-order, so semantics are identical)."""
    ctr = 0
    for fn in nc.m.functions:
        for blk in fn.blocks:
            out = []
            for inst in blk.instructions:
                si = inst.sync_info
                if si is not None and si.on_wait and len(si.on_wait) > 1:
                    waits = list(si.on_wait)
                    for w in waits[:-1]:
                        nop = mybir.InstNoOp(name=f"wsplit_{ctr}", ins=[], outs=[])
                        ctr += 1
                        nop.engine = inst.engine
                        nop.sync_info = mybir.SyncInfo(on_wait=[w], on_update=[])
                        out.append(nop)
                    inst.sync_info = mybir.SyncInfo(
                        on_wait=[waits[-1]], on_update=list(si.on_update)
                    )
                out.append(inst)
            blk.instructions = out


def build_kernel():
    nc = bass.Bass()

    aT_d = nc.dram_tensor("aT", [D, L], BF16, kind="ExternalInput")
    bT_d = nc.dram_tensor("bT", [D, L], BF16, kind="ExternalInput")
    ao_d = nc.dram_tensor("ao", [L, DO], BF16, kind="ExternalInput")
    bo_d = nc.dram_tensor("bo", [L, DO], BF16, kind="ExternalInput")
    w_d = nc.dram_tensor("w", [D, D], BF16, kind="ExternalInput")
    wt_d = nc.dram_tensor("wt", [D, D], BF16, kind="ExternalInput")
    bias_d = nc.dram_tensor("bias", [D, 1], FP32, kind="ExternalInput")
    bias_t_d = nc.dram_tensor("bias_t", [D, 1], FP32, kind="ExternalInput")
    fa_d = nc.dram_tensor("feature_a", [L, D], FP32, kind="ExternalOutput")
    fb_d = nc.dram_tensor("feature_b", [L, D], FP32, kind="ExternalOutput")

    # DRAM views for chunked access
    aT_v = aT_d[:].rearrange("(kc p) l -> p kc l", p=P)      # [128, KD, L]
    bT_v = bT_d[:].rearrange("(kc p) l -> p kc l", p=P)
    ao_v = ao_d[:].rearrange("(n p) c -> p n c", p=P)        # [128, NL, 257]
    bo_v = bo_d[:].rearrange("(n p) c -> p n c", p=P)
    w_v = w_d[:].rearrange("(kc p) n -> p kc n", p=P)        # [128, KD, D]
    wt_v = wt_d[:].rearrange("(kc p) n -> p kc n", p=P)
    bias_v = bias_d[:].rearrange("(c p) one -> p c one", p=P)
    bias_t_v = bias_t_d[:].rearrange("(c p) one -> p c one", p=P)
    fa_v = fa_d[:].rearrange("(n p) d -> p n d", p=P)
    fb_v = fb_d[:].rearrange("(n p) d -> p n d", p=P)

    with SplitDrainTileContext(nc) as tc:
        with (
            tc.tile_pool(name="consts", bufs=1) as consts,
            tc.tile_pool(name="bigbuf", bufs=1) as bigbuf,
            tc.tile_pool(name="etile", bufs=5) as etile,
            tc.tile_pool(name="stage", bufs=4) as stage,
            tc.tile_pool(name="ps_s", bufs=2, space="PSUM") as ps_s_pool,
            tc.tile_pool(name="ps_t", bufs=2, space="PSUM") as ps_t_pool,
            tc.tile_pool(name="ps_fb", bufs=1, space="PSUM") as ps_fb_pool,
            tc.tile_pool(name="warm", bufs=1) as warm_pool,
        ):
            # ---- PE warmup: dummy matmuls so the HAM clock-gate opens
            #      before the real stream begins ----
            wsrc = warm_pool.tile([P, P], BF16)
            nc.vector.memset(wsrc[:], 0.0)
            # preload the exp/relu ACT table set while ACT is idle
            wact = warm_pool.tile([P, 1], FP32)
            nc.scalar.activation(out=wact[:], in_=wsrc[:, 0:1], func=EXP)
            ps_w = ps_s_pool.tile([P, F], FP32, name="ps_w", tag="ps")
            for _ in range(48):
                nc.tensor.matmul(ps_w[:, :P], lhsT=wsrc[:], rhs=wsrc[:],
                                 start=True, stop=True)

            # ---- constants ----
            w_sb = consts.tile([P, KD, D], BF16)
            nc.gpsimd.dma_start(out=w_sb[:], in_=w_v)
            wt_sb = consts.tile([P, KD, D], BF16)
            nc.gpsimd.dma_start(out=wt_sb[:], in_=wt_v)
            bias_sb = consts.tile([P, KD], FP32)
            nc.gpsimd.dma_start(out=bias_sb[:], in_=bias_v[:, :, 0])
            bias_t_sb = consts.tile([P, KD], FP32)
            nc.gpsimd.dma_start(out=bias_t_sb[:], in_=bias_t_v[:, :, 0])
            ident = consts.tile([P, P], BF16)
            make_identity(nc, ident[:])

            # ---- big SBUF residents ----
            aT_sb = bigbuf.tile([P, KD, L], BF16)
            bT_sb = bigbuf.tile([P, KD, L], BF16)
            ao_sb = bigbuf.tile([P, NL, DO], BF16)   # [a | 1]
            bo_sb = bigbuf.tile([P, NL, DO], BF16)   # [b | 1]
            at_a = bigbuf.tile([P, KD, L], BF16)     # relu(a@(W*temp))^T
            at_b = bigbuf.tile([P, KD, L], BF16)     # relu(b@W)^T
            et_sb = bigbuf.tile([P, NL, L], BF16)    # E^T resident [m, mc, la]

            # ---- input DMAs: spread across SP-HWDGE / ACT-HWDGE / SWDGE.
            # bT slice 0 first (dense_b starts), then all of aT (scores need
            # at_a progressively), ao early (fb rhs), rest of bT, bo last
            # (only needed in phase B).
            W2 = F
            nc.sync.dma_start(out=bT_sb[:, 0, 0:W2], in_=bT_v[:, 0, 0:W2])
            nc.sync.dma_start(out=bT_sb[:, 1, 0:W2], in_=bT_v[:, 1, 0:W2])
            for hf in range(L // W2):
                sl = slice(hf * W2, (hf + 1) * W2)
                for kc in range(KD):
                    eng = nc.scalar if hf % 2 == 0 else nc.sync
                    eng.dma_start(out=aT_sb[:, kc, sl], in_=aT_v[:, kc, sl])
            for q in range(4):
                sl = slice(q * 4, (q + 1) * 4)
                nc.gpsimd.dma_start(out=ao_sb[:, sl, :], in_=ao_v[:, sl, :])
            for hf in range(1, L // W2):
                sl = slice(hf * W2, (hf + 1) * W2)
                for kc in range(KD):
                    nc.sync.dma_start(out=bT_sb[:, kc, sl], in_=bT_v[:, kc, sl])
            for q in range(4):
                sl = slice(q * 4, (q + 1) * 4)
                nc.gpsimd.dma_start(out=bo_sb[:, sl, :], in_=bo_v[:, sl, :])

            # ---- phase 0: dense + relu ----
            def dense_block(src_sb, dst, ls, w_tile, b_tile, act_engine):
                sl = slice(ls * F, (ls + 1) * F)
                for dout in range(KD):
                    wcol = slice(dout * P, (dout + 1) * P)
                    ps = ps_s_pool.tile([P, F], FP32, name="ps", tag="ps")
                    for kc in range(KD):
                        nc.tensor.matmul(
                            ps[:],
                            lhsT=w_tile[:, kc, wcol],
                            rhs=src_sb[:, kc, sl],
                            start=(kc == 0),
                            stop=(kc == KD - 1),
                        )
                    if act_engine == "act":
                        nc.scalar.activation(
                            out=dst[:, dout, sl], in_=ps[:], func=RELU,
                            bias=b_tile[:, dout : dout + 1],
                        )
                    else:
                        # relu(x + bias) fused on the vector engine
                        nc.vector.tensor_scalar(
                            out=dst[:, dout, sl], in0=ps[:],
                            scalar1=b_tile[:, dout : dout + 1], scalar2=0.0,
                            op0=mybir.AluOpType.add, op1=mybir.AluOpType.max,
                        )

            # dense_b slice 0 first (so scores for ms=0 can begin), then all
            # of dense_a progressively, then the remaining dense_b slices
            dense_block(bT_sb, at_b, 0, w_sb, bias_sb, "dve")
            for ls in range(NS):
                dense_block(aT_sb, at_a, ls, wt_sb, bias_t_sb, "act")
            for ls in range(1, NS):
                dense_block(bT_sb, at_b, ls, w_sb, bias_sb, "dve")

            # ---- phase A: E tiles [la:128, m:512] once; fb accum + colsum
            #      via ones-column; transpose into et_sb ----
            # Software-pipelined: fb/transpose of tile i-2 run behind the
            # score matmuls of tile i so the PE never waits on ACT's exp.
            fb_psums = {}   # ms -> list of 4 psum handles
            pend = []       # [(ms, lc, e_tile), ...] awaiting fb+transpose

            def flush_one():
                ms, lc, e = pend.pop(0)
                for j in range(4):
                    nc.tensor.matmul(
                        fb_psums[ms][j],
                        lhsT=e[:, j * P : (j + 1) * P],
                        rhs=ao_sb[:, lc, :],
                        start=(lc == 0),
                        stop=(lc == NL - 1),
                    )
                ps_t = ps_t_pool.tile([P, F], BF16, name="ps_t", tag="pst")
                for j in range(4):
                    nc.tensor.transpose(
                        ps_t[:, j * P : (j + 1) * P],
                        e[:, j * P : (j + 1) * P],
                        ident[:],
                    )
                nc.vector.tensor_copy(
                    out=et_sb[:, ms * 4 : ms * 4 + 4, lc * P : (lc + 1) * P],
                    in_=ps_t[:].rearrange("p (j q) -> p j q", j=4),
                )
                if lc == NL - 1:
                    finish_ms(ms)

            def finish_ms(ms):
                # normalize feature_b chunks of this ms and DMA them out;
                # j=0 first so its bank frees before the next ms needs it
                for j in range(4):
                    mc = ms * 4 + j
                    ps = fb_psums[ms][j]
                    rv = stage.tile([P, 1], FP32, name="rv", tag="rv")
                    nc.vector.reciprocal(rv[:], ps[:, D : D + 1])
                    fb_t = stage.tile([P, D], FP32, name="fb_t", tag="fb_t")
                    nc.vector.tensor_scalar_mul(
                        out=fb_t[:], in0=ps[:, 0:D], scalar1=rv[:]
                    )
                    eng = nc.scalar if j % 2 else nc.sync
                    eng.dma_start(out=fb_v[:, mc, :], in_=fb_t[:])
                del fb_psums[ms]

            for ms in range(NS):
                msl = slice(ms * F, (ms + 1) * F)
                fb_psums[ms] = [
                    ps_fb_pool.tile([P, DO], FP32, name=f"psfb{j}", tag=f"psfb{j}")
                    for j in range(4)
                ]
                for lc in range(NL):
                    lsl = slice(lc * P, (lc + 1) * P)
                    ps = ps_s_pool.tile([P, F], FP32, name="ps", tag="ps")
                    for kc in range(KD):
                        nc.tensor.matmul(
                            ps[:],
                            lhsT=at_a[:, kc, lsl],
                            rhs=at_b[:, kc, msl],
                            start=(kc == 0),
                            stop=(kc == KD - 1),
                        )
                    e = etile.tile([P, F], BF16, name="et", tag="et")
                    nc.scalar.activation(out=e[:], in_=ps[:], func=EXP)
                    pend.append((ms, lc, e))
                    if len(pend) > 2:
                        flush_one()
            while pend:
                flush_one()

            # ---- phase B: feature_a from et_sb; rowsum via ones-column ----
            for c in range(NL):
                csl = slice(c * P, (c + 1) * P)
                ps_fa = ps_s_pool.tile([P, DO], FP32, name="ps_fa", tag="ps")
                for mc in range(NL):
                    nc.tensor.matmul(
                        ps_fa[:],
                        lhsT=et_sb[:, mc, csl],
                        rhs=bo_sb[:, mc, :],
                        start=(mc == 0),
                        stop=(mc == NL - 1),
                    )
                rv = stage.tile([P, 1], FP32, name="rva", tag="rv")
                nc.vector.reciprocal(rv[:], ps_fa[:, D : D + 1])
                fa_t = stage.tile([P, D], FP32, name="fa_t", tag="fb_t")
                nc.vector.tensor_scalar_mul(
                    out=fa_t[:], in0=ps_fa[:, 0:D], scalar1=rv[:]
                )
                eng = nc.scalar if c % 2 else nc.sync
                eng.dma_start(out=fa_v[:, c, :], in_=fa_t[:])

    split_multiwaits(nc)
    return nc


_NC_CACHE = {}


def make_in_maps(a, b, dense_w, dense_b, temp):
    bf = ml_dtypes.bfloat16
    w_arr = np.ascontiguousarray(dense_w.astype(bf))
    wt_arr = np.ascontiguousarray((dense_w * temp).astype(bf))
    bias_arr = np.ascontiguousarray(dense_b.reshape(D, 1).astype(np.float32))
    bias_t_arr = np.ascontiguousarray((dense_b * temp).reshape(D, 1).astype(np.float32))
    ones = np.ones((L, 1), np.float32)
    in_maps = []
    for i in range(B):
        ao = np.concatenate([a[i], ones], axis=1).astype(bf)
        bo = np.concatenate([b[i], ones], axis=1).astype(bf)
        in_maps.append({
            "aT": np.ascontiguousarray(a[i].T.astype(bf)),
            "bT": np.ascontiguousarray(b[i].T.astype(bf)),
            "ao": np.ascontiguousarray(ao),
            "bo": np.ascontiguousarray(bo),
            "w": w_arr,
            "wt": wt_arr,
            "bias": bias_arr,
            "bias_t": bias_t_arr,
        })
    return in_maps


def run(a, b, dense_w, dense_b, temperature, **spmd_kwargs):
    a = np.asarray(a, dtype=np.float32)
    b = np.asarray(b, dtype=np.float32)
    dense_w = np.asarray(dense_w, dtype=np.float32)
    dense_b = np.asarray(dense_b, dtype=np.float32)
    temp = np.float32(np.asarray(temperature).reshape(-1)[0])

    if "nc" not in _NC_CACHE:
        _NC_CACHE["nc"] = build_kernel()
    nc = _NC_CACHE["nc"]

    in_maps = make_in_maps(a, b, dense_w, dense_b, temp)
    res = run_bass_kernel_spmd(nc, in_maps, core_ids=list(range(B)), **spmd_kwargs)
    fa = np.stack([res.results[i]["feature_a"] for i in range(B)])
    fb = np.stack([res.results[i]["feature_b"] for i in range(B)])
    return fa, fb, res


def kernel(a, b, mask_a, mask_b, dense_w, dense_b, temperature, **_ignored):
    fa, fb, _ = run(a, b, dense_w, dense_b, temperature)
    return fa, fb


if __name__ == "__main__":
    rng = np.random.default_rng(0)
    a = rng.standard_normal((B, L, D), dtype=np.float32)
    b = rng.standard_normal((B, L, D), dtype=np.float32)
    w = (rng.standard_normal((D, D)) / 16).astype(np.float32)
    bias = np.zeros((D,), np.float32)
    fa, fb = kernel(a, b, None, None, w, bias, np.float32(1 / 16))
    print(fa.shape, fb.shape, fa.dtype)
